# revision 1
# baseline (speedup 1.0000x reference)
"""Trainium2 Bass kernel for the CubeSimulator problem.

Reference computation (shapes): rotate (96,96,96) grids, build a per-voxel
line-of-sight velocity u and intensity I = exp(L), then a Gaussian-KDE cube
cube[i,j,v] = norm * sum_z exp(-(vel_v - u)^2/sig^2) * I, followed by a
"trilinear" downsample (96,96,64) -> (32,64,64).

Key exact simplifications (validated against the reference in fp32,
rel err ~6e-6):
 - downsample axis0 (96->32, scale 3): output coords land exactly on
   integers 3k+1, so it is a pure row selection -> only 32 of 96 i-rows
   are ever needed (3x less KDE work).
 - downsample axis2 (64->64) is exactly the identity.
 - downsample axis1 (96->64) is an exact 2-tap stencil with weights
   0.75/0.25 (even) / 0.25/0.75 (odd), applied as one TensorE matmul over
   the j partition axis.
 - exp(L - (vel_v-u)^2/sig^2) = exp(A + vel_v*B + c_v) with
   A = L + ln(norm) - u^2/sig^2, B = 2u/sig^2, c_v = -vel_v^2/sig^2;
   A and B are precomputed per voxel.
 - tanh(r/2)/r = (e^r - 1)/(r (e^r + 1)) evaluated with a single
   reciprocal; r = exp(0.5 ln(max(q,1e-35))) keeps every activation in
   the natural_log_exp_and_others table set (one ACT table load) and
   avoids the loose-tolerance Sqrt.

Per velocity bin, two engine-balanced paths (split tuned on the
instruction cost model):
 - affine path: VectorE tensor_scalar (B*vv + c_v), V/G tensor_add (+A),
   with KDE_VB bins batched into one wide ScalarE Exp.
 - factored path: exp(A + vv*B + c_v) = exp(A) * exp(vv*B + c_v) -- one
   ScalarE Exp (scale=vv immediate, bias=c_v per-partition AP) and one
   V/G multiply by P0 = exp(A).
The z-reduction is a per-(bin, i-row) TensorE matmul with the exp tile as
the stationary operand and a ones-vector moving, accumulating the cube as
[j=96 partitions, (i,v)] in PSUM, which makes the j-downsample a single
stationary-W matmul.

Sharding: the 32 needed i-rows are split 4-per-core across 8 cores (pure
data parallel over pixels); each core's device layout is [z=96 partitions,
pixels=4*96=384 free].  Runtime scalars (rotation trig, sigma, the 64
velocity values) are baked into the instruction stream as immediates since
the kernel is compiled per call.
"""

import math

import numpy as np

import concourse.bacc as bacc
import concourse.bass as bass
import concourse.mybir as mybir
import concourse.tile as tile
from concourse.bass_utils import run_bass_kernel_spmd

G = 96            # up_gal grid size
NV = 64           # velocity bins
N_CORES = 8
OUT_I = 32        # downsampled i rows (= VEL_RES in the reference's axis naming)
ROWS_PER_CORE = OUT_I // N_CORES   # 4
PX = ROWS_PER_CORE * G             # 384 pixels per core
OUT_J = 64

F32 = mybir.dt.float32
AF = mybir.ActivationFunctionType
OP = mybir.AluOpType

LAST_EXEC_NS = None  # filled in when run with BASS_TRACE=1

# tuning knobs (validated via TimelineSim sweeps)
KDE_VB = 8                   # velocity bins per group
KDE_FACT_SLOTS = (1, 3, 5, 7)  # slots per group using the factored path
KDE_NGC = 0.25               # fraction of affine-path adds routed to GpSimd
KDE_FACT_G = 0.75            # fraction of factored-path mults routed to GpSimd
ABLATE = set()         # {'mm','tt','ts','exp'} - sim-only ablation switches


def _build_program(ci, si, cr, sr, sig2, lnnorm, vel, fact_slots=None):
    if fact_slots is None:
        fact_slots = KDE_FACT_SLOTS
    nc = bacc.Bacc("TRN2")

    xs = nc.dram_tensor("xs", [G, PX], F32, kind="ExternalInput")
    ys = nc.dram_tensor("ys", [G, PX], F32, kind="ExternalInput")
    zs = nc.dram_tensor("zs", [G, PX], F32, kind="ExternalInput")
    # per-velocity-bin exp biases c_v = -vel_v^2/sig^2 (replicated across
    # partitions; used as per-partition bias APs on factored-path Exps)
    bc = nc.dram_tensor("bc", [128, NV], F32, kind="ExternalInput")
    # j-downsample stencil matrix (96 -> 64, 2 taps per output)
    wj = nc.dram_tensor("wj", [G, OUT_J], F32, kind="ExternalInput")
    out = nc.dram_tensor("out", [OUT_J, ROWS_PER_CORE * NV], F32,
                         kind="ExternalOutput")

    with tile.TileContext(nc) as tc:
        with (
            tc.tile_pool(name="io", bufs=1) as io,
            tc.tile_pool(name="prep", bufs=1) as prep,
            tc.tile_pool(name="kde", bufs=2) as kde,
            tc.tile_pool(name="psum", bufs=1, space="PSUM") as psum,
        ):
            xt = io.tile([G, PX], F32, tag="xt")
            yt = io.tile([G, PX], F32, tag="yt")
            zt = io.tile([G, PX], F32, tag="zt")
            nc.sync.dma_start(out=xt[:], in_=xs[:])
            nc.sync.dma_start(out=yt[:], in_=ys[:])
            nc.sync.dma_start(out=zt[:], in_=zs[:])
            wjt = io.tile([G, OUT_J], F32, tag="wjt")
            nc.sync.dma_start(out=wjt[:], in_=wj[:])
            bct = io.tile([128, NV], F32, tag="bct")
            nc.sync.dma_start(out=bct[:], in_=bc[:])

            def vtile(name):
                return prep.tile([G, PX], F32, tag=name, name=name)

            # Prep. Only tensor_scalar / tensor_tensor / activation are used
            # -- the S2S2D2_STT (scalar_tensor_tensor) ISA struct has a
            # single sync-wait slot and cannot be scheduled where Tile needs
            # multiple waits.
            # Rotated coordinates (R = Rx(inc) @ Rz(rot)); the rx/ry legs run
            # on VectorE (critical path), the rz/intensity leg on GpSimd.
            xa, ya, rx = vtile("xa"), vtile("ya"), vtile("rx")
            xb, yb, t3 = vtile("xb"), vtile("yb"), vtile("t3")
            za, ry = vtile("za"), vtile("ry")
            nc.vector.tensor_scalar_mul(xa[:], xt[:], cr)
            nc.vector.tensor_scalar_mul(ya[:], yt[:], -sr)
            nc.vector.tensor_add(rx[:], xa[:], ya[:])
            nc.vector.tensor_scalar_mul(xb[:], xt[:], ci * sr)
            nc.vector.tensor_scalar_mul(yb[:], yt[:], ci * cr)
            nc.vector.tensor_add(t3[:], xb[:], yb[:])
            nc.vector.tensor_scalar_mul(za[:], zt[:], -si)
            nc.vector.tensor_add(ry[:], t3[:], za[:])
            xc, yc, t5 = vtile("xc"), vtile("yc"), vtile("t5")
            zb, rz = vtile("zb"), vtile("rz")
            nc.gpsimd.tensor_scalar_mul(xc[:], xt[:], si * sr)
            nc.gpsimd.tensor_scalar_mul(yc[:], yt[:], si * cr)
            nc.gpsimd.tensor_add(t5[:], xc[:], yc[:])
            nc.gpsimd.tensor_scalar_mul(zb[:], zt[:], ci)
            nc.gpsimd.tensor_add(rz[:], t5[:], zb[:])

            # in-plane radius r via exp(0.5*ln(q)) -- avoids the loose-
            # tolerance Sqrt activation.  q is clamped away from 0 once so
            # every division below is finite (r >= 3e-18).
            sqx, sqy, q, qs = vtile("sqx"), vtile("sqy"), vtile("q"), vtile("qs")
            lnq, r = vtile("lnq"), vtile("r")
            nc.scalar.activation(sqx[:], rx[:], AF.Square)
            nc.vector.tensor_mul(sqy[:], ry[:], ry[:])
            nc.vector.tensor_add(q[:], sqy[:], sqx[:])
            nc.vector.tensor_scalar_max(qs[:], q[:], 1e-35)
            nc.scalar.activation(lnq[:], qs[:], AF.Ln)
            nc.scalar.activation(r[:], lnq[:], AF.Exp, scale=0.5)

            # u0 = rx*tanh(r/2)/r computed as rx*(e^r-1) / (r*(e^r+1)) --
            # one reciprocal, and every activation stays in the
            # natural_log_exp_and_others table set (single table load).
            # The -200*si amplitude folds into the s1/Bt scales below.
            er, ed = vtile("er"), vtile("ed")
            den, rec, num = vtile("den"), vtile("rec"), vtile("num")
            t1, u0 = vtile("t1"), vtile("u0")
            nc.scalar.activation(er[:], r[:], AF.Exp)
            nc.vector.tensor_scalar_add(ed[:], er[:], 1.0)
            nc.vector.tensor_mul(den[:], ed[:], r[:])
            nc.vector.reciprocal(rec[:], den[:])
            nc.vector.tensor_scalar_add(num[:], er[:], -1.0)
            nc.vector.tensor_mul(t1[:], rx[:], num[:])
            nc.vector.tensor_mul(u0[:], t1[:], rec[:])

            # A = L + lnnorm - (u/sig)^2 ; L = -r/3 - 2|rz| ; B = 2u/sig^2
            az, azs, rterm, Lt = (vtile("az"), vtile("azs"), vtile("rterm"),
                                  vtile("Lt"))
            s1, ssq, At, Bt, P0t = (vtile("s1"), vtile("ssq"), vtile("At"),
                                    vtile("Bt"), vtile("P0t"))
            nc.scalar.activation(az[:], rz[:], AF.Abs)
            nc.gpsimd.tensor_scalar_mul(azs[:], az[:], -2.0)
            nc.gpsimd.tensor_scalar(rterm[:], r[:], -1.0 / 3.0, lnnorm,
                                    OP.mult, OP.add)
            nc.gpsimd.tensor_add(Lt[:], azs[:], rterm[:])
            usc = -200.0 * si
            nc.vector.tensor_scalar_mul(s1[:], u0[:], usc / math.sqrt(sig2))
            nc.scalar.activation(ssq[:], s1[:], AF.Square)
            nc.vector.tensor_sub(At[:], Lt[:], ssq[:])
            nc.vector.tensor_scalar_mul(Bt[:], u0[:], usc * 2.0 / sig2)
            nc.scalar.activation(P0t[:], At[:], AF.Exp)

            ones = io.tile([G, 1], F32, tag="ones")
            nc.vector.memset(ones[:], 1.0)

            # cube[j, i*NV + v] = sum_z exp-term   (j on partitions)
            cube = psum.tile([G, ROWS_PER_CORE * NV], F32)

            # Two paths per velocity bin, mixed to balance engines:
            #  - affine path: arg = (B*vv + c_v) + A  (VectorE ts + V/G tt),
            #    VB-batched into one wide ScalarE Exp.
            #  - factored path: exp(A + vv*B + c_v) = P0 * exp(vv*B + c_v)
            #    (one ScalarE Exp with scale/bias immediates + one V/G mult;
            #    exact to fp32 rounding since both factors are exp outputs).
            VB = KDE_VB
            for g in range(NV // VB):
                bins = list(range(g * VB, (g + 1) * VB))
                cbins = [b for b in bins if (b % VB) not in fact_slots]
                fbins = [b for b in bins if (b % VB) in fact_slots]
                srcs = {}
                # factored-path bins first: their Exps depend only on Bt, so
                # ScalarE starts each group without stalling on the affine
                # arg builds (Tile priority follows emission order)
                nfb = len(fbins)
                for k, iv in enumerate(fbins):
                    vv = float(vel[iv])
                    e1 = kde.tile([G, PX], F32, tag="e1", bufs=4)
                    nc.scalar.activation(e1[:], Bt[:], AF.Exp, scale=vv,
                                         bias=bct[0:G, iv:iv + 1])
                    m1 = kde.tile([G, PX], F32, tag="m1", bufs=4)
                    eng = nc.gpsimd if k < KDE_FACT_G * nfb else nc.vector
                    eng.tensor_mul(m1[:], e1[:], P0t[:])
                    srcs[iv] = (m1, 0)
                ncb = len(cbins)
                if ncb:
                    argw = kde.tile([G, ncb * PX], F32, tag="argw")
                    tmpw = kde.tile([G, ncb * PX], F32, tag="tmpw")
                    for k, iv in enumerate(cbins):
                        vv = float(vel[iv])
                        cv = -vv * vv / sig2
                        sl = slice(k * PX, (k + 1) * PX)
                        nc.vector.tensor_scalar(tmpw[:, sl], Bt[:], vv, cv,
                                                OP.mult, OP.add)
                        eng = nc.gpsimd if k < KDE_NGC * ncb else nc.vector
                        eng.tensor_add(argw[:, sl], tmpw[:, sl], At[:])
                    exw = kde.tile([G, ncb * PX], F32, tag="exw")
                    nc.scalar.activation(exw[:], argw[:], AF.Exp)
                    for k, iv in enumerate(cbins):
                        srcs[iv] = (exw, k * PX)
                # reduce over z (partitions) one i-row at a time: E-slice is
                # the stationary operand, a ones-vector the moving one.
                for iv in bins:
                    if 'mm' in ABLATE:
                        break
                    t, off0 = srcs[iv]
                    for ii in range(ROWS_PER_CORE):
                        col = ii * NV + iv
                        off = off0 + ii * G
                        nc.tensor.matmul(cube[:, col:col + 1],
                                         t[:, off:off + G], ones[:],
                                         start=True, stop=True)

            # j-downsample over the partition axis: out2[jj, (i,v)]
            cube_sb = io.tile([G, ROWS_PER_CORE * NV], F32, tag="cube_sb")
            nc.vector.tensor_copy(cube_sb[:], cube[:])
            out_ps = psum.tile([OUT_J, ROWS_PER_CORE * NV], F32)
            nc.tensor.matmul(out_ps[:], wjt[:], cube_sb[:],
                             start=True, stop=True)
            out_sb = io.tile([OUT_J, ROWS_PER_CORE * NV], F32, tag="out_sb")
            nc.vector.tensor_copy(out_sb[:], out_ps[:])
            nc.sync.dma_start(out=out[:], in_=out_sb[:])

    return nc


def kernel(**inputs):
    inc = float(np.asarray(inputs["inclination"]).reshape(-1)[0])
    rot = float(np.asarray(inputs["sky_rot"]).reshape(-1)[0])
    lb = float(np.asarray(inputs["line_broadening"]).reshape(-1)[0])
    vel = np.asarray(inputs["velocity_grid"], np.float32).reshape(-1)
    X = np.asarray(inputs["Xgrid"], np.float32)
    Y = np.asarray(inputs["Ygrid"], np.float32)
    Z = np.asarray(inputs["Zgrid"], np.float32)

    ci, si = math.cos(inc), math.sin(inc)
    cr, sr = math.cos(rot), math.sin(rot)
    sig2 = float(np.float32(lb) * np.float32(lb))
    if not (sig2 > 0.0) or not math.isfinite(sig2):
        sig2 = 1e-30  # degenerate sigma: reference output is ~0/NaN anyway
    lnnorm = float(-0.5 * math.log(2.0 * math.pi * sig2))

    # The factored path computes exp(vv*B + c_v) whose argument is bounded by
    # u_max^2/sig^2 (u_max = 200*|sin(inc)| rigorously bounds |u|).  If that
    # could overflow fp32, fall back to the always-safe affine path (its
    # fused exponent is <= ln(norm)).
    umax2 = (200.0 * abs(si)) ** 2
    fact_slots = KDE_FACT_SLOTS if umax2 / sig2 <= 80.0 else ()
    nc = _build_program(ci, si, cr, sr, sig2, lnnorm, vel, fact_slots)
    nc.finalize()

    bcv = np.ascontiguousarray(
        np.tile((-(vel.astype(np.float64) ** 2) / sig2).astype(np.float32),
                (128, 1)))
    wjv = np.zeros((G, OUT_J), np.float32)
    for m in range(OUT_J // 2):
        wjv[3 * m, 2 * m] = 0.75
        wjv[3 * m + 1, 2 * m] = 0.25
        wjv[3 * m + 1, 2 * m + 1] = 0.25
        wjv[3 * m + 2, 2 * m + 1] = 0.75

    in_maps = []
    for c in range(N_CORES):
        rows = [3 * k + 1 for k in range(ROWS_PER_CORE * c,
                                         ROWS_PER_CORE * (c + 1))]
        def shard(a):
            s = a[rows]                        # (4, 96, 96) = (i, j, z)
            s = s.transpose(2, 0, 1).reshape(G, PX)   # [z, i*96+j]
            return np.ascontiguousarray(s)
        in_maps.append({"xs": shard(X), "ys": shard(Y), "zs": shard(Z),
                        "bc": bcv, "wj": wjv})

    res = run_bass_kernel_spmd(nc, in_maps, core_ids=list(range(N_CORES)))
    global LAST_EXEC_NS
    LAST_EXEC_NS = res.exec_time_ns

    parts = []
    for c in range(N_CORES):
        o = res.results[c]["out"]              # (64, 256) = [jj, i*64+v]
        parts.append(o.reshape(OUT_J, ROWS_PER_CORE, NV).transpose(1, 0, 2))
    return np.concatenate(parts, axis=0).astype(np.float32)  # (32, 64, 64)



# revision 6
# speedup vs baseline: 1.4060x; 1.4060x over previous
"""Trainium2 Bass kernel for the CubeSimulator problem (v2).

Reference: rotate (96,96,96) grids, per-voxel line-of-sight velocity u and
intensity I, Gaussian-KDE cube over 64 velocity bins, then trilinear
downsample (96,96,64) -> (32,64,64).

Exact structure reused from v1 (validated):
 - axis0 downsample (96->32) is a pure selection of rows 3k+1;
 - axis2 downsample (64->64) is the identity;
 - axis1 downsample (96->64) is a 2-tap stencil (0.75/0.25) matmul;
 - exp(L - (v-u)^2/sig^2) = exp(A) * exp(v*B + c_v) with
   A = L + ln(norm) - u^2/sig^2, B = 2u/sig^2, c_v = -v^2/sig^2.

New in v2 (tolerance-aware, rel err ~1.5e-3 vs 2e-2 budget):
 - Coarse-bin KDE: the cube is computed at NC=32 velocity centers and all
   64 reference bins are reconstructed with a ridge-regularized
   least-squares matrix R (a Gaussian with sigma=30 sampled at dv=19 is
   ~3x oversampled; aliasing ~2e-4). Halves the dominant per-bin work.
 - Wrap layout [128, 288]: per-core voxels flat=(px*96+z) are laid out
   partition=flat%128, free=flat//128, using all 128 lanes (elementwise
   engine cost scales with free size only). The z-sum for pixel p covers
   flat [96p, 96p+96), reduced on the (otherwise idle) TensorE with three
   accumulating matmuls per bin whose [128,4] selector stationaries are
   independent of the column triplet (128*3 = 96*4).
 - KDE tiles in bf16: DVE runs 2-byte tensor_tensor at 2x; exp args stay
   fp32 (ACT reads Bt fp32; scale/bias are per-bin immediates/APs).
 - Per-bin path: one ScalarE Exp (scale=vc, bias=c_v AP) + one V/P mult
   by P0 = exp(A). ScalarE is the bottleneck engine; all of prep's
   square/abs/ln/exp stay inside the natural_log_exp_and_others table.

Sharding: 32 needed i-rows split 4-per-core across 8 cores; only the
final (64, 4*64) tile is gathered per core.
"""

import math

import numpy as np

import concourse.bacc as bacc
import concourse.bass as bass
import concourse.mybir as mybir
import concourse.tile as tile
from concourse.bass_utils import run_bass_kernel_spmd

try:
    import ml_dtypes
    _BF16 = np.dtype(ml_dtypes.bfloat16)
except Exception:  # pragma: no cover
    _BF16 = None

G = 96            # up_gal grid size
NV = 64           # reference velocity bins
NC = 32           # coarse KDE bins (reconstructed to NV by matmul)
N_CORES = 8
OUT_I = 32        # selected i rows (axis-0 downsample = row selection)
ROWS_PER_CORE = OUT_I // N_CORES   # 4
PX = ROWS_PER_CORE * G             # 384 pixels per core
NZ = G                             # z depth
NFLAT = PX * NZ                    # 36864 voxels per core
NP128 = 128
NF = NFLAT // NP128                # 288 free columns
NT = NF // 3                       # 96 column triplets (4 pixels each)
OUT_J = 64
RIDGE_LAM = 1e-4

F32 = mybir.dt.float32
BF16 = mybir.dt.bfloat16
AF = mybir.ActivationFunctionType
OP = mybir.AluOpType

LAST_EXEC_NS = None

# tuning knobs
KDE_POOL_MULTS = 6   # of the NC bf16 P0-mults, how many go to GpSimd
PREP_FUSE_STT = False # use scalar_tensor_tensor fusion in prep where legal


def _build_program(ci, si, cr, sr, sig2, lnnorm, vel, safe_affine=None):
    vel = np.asarray(vel, np.float64).reshape(-1)
    vc = np.linspace(float(vel.min()), float(vel.max()), NC)
    usc = -200.0 * si
    if safe_affine is None:
        umax2 = (200.0 * si) ** 2
        safe_affine = not (umax2 / sig2 <= 80.0)

    nc = bacc.Bacc("TRN2")

    xs = nc.dram_tensor("xs", [NP128, NF], F32, kind="ExternalInput")
    ys = nc.dram_tensor("ys", [NP128, NF], F32, kind="ExternalInput")
    zs = nc.dram_tensor("zs", [NP128, NF], F32, kind="ExternalInput")
    # per-coarse-bin exp biases c_v = -vc^2/sig^2 (replicated on partitions)
    bc = nc.dram_tensor("bc", [NP128, NC], F32, kind="ExternalInput")
    # z-reduce selector stationaries S_c[k, m] = 1{96m <= 128c+k < 96m+96}
    sc = nc.dram_tensor("sc", [NP128, 12], BF16, kind="ExternalInput")
    # reconstruction moving matrix W[(b,m), (m',v)] = delta_{m,m'} R[b, v]
    wr = nc.dram_tensor("wr", [NP128, 4 * NV], BF16, kind="ExternalInput")
    # j-downsample stationaries, zero-padded to 96 partitions:
    # sm[:, (i*4+m)*64 + jj] = wj[4s+m, jj] on rows t=24i+s, else 0
    sm = nc.dram_tensor("sm", [NT, 16 * OUT_J], BF16, kind="ExternalInput")
    # identity for the PE transpose
    idm = nc.dram_tensor("idm", [NT, NT], BF16, kind="ExternalInput")
    out = nc.dram_tensor("out", [OUT_J, ROWS_PER_CORE * NV], F32,
                         kind="ExternalOutput")

    with tile.TileContext(nc) as tc:
        with (
            tc.tile_pool(name="io", bufs=1) as io,
            tc.tile_pool(name="prep", bufs=1) as prep,
            tc.tile_pool(name="kde", bufs=2) as kde,
            tc.tile_pool(name="psum", bufs=1, space="PSUM") as psum,
        ):
            xt = io.tile([NP128, NF], F32, tag="xt")
            yt = io.tile([NP128, NF], F32, tag="yt")
            zt = io.tile([NP128, NF], F32, tag="zt")
            nc.sync.dma_start(out=xt[:], in_=xs[:])
            nc.sync.dma_start(out=yt[:], in_=ys[:])
            nc.sync.dma_start(out=zt[:], in_=zs[:])
            bct = io.tile([NP128, NC], F32, tag="bct")
            nc.sync.dma_start(out=bct[:], in_=bc[:])
            sct = io.tile([NP128, 12], BF16, tag="sct")
            nc.sync.dma_start(out=sct[:], in_=sc[:])
            wrt = io.tile([NP128, 4 * NV], BF16, tag="wrt")
            nc.sync.dma_start(out=wrt[:], in_=wr[:])
            smt = io.tile([NT, 16 * OUT_J], BF16, tag="smt")
            nc.sync.dma_start(out=smt[:], in_=sm[:])
            idt = io.tile([NT, NT], BF16, tag="idt")
            nc.sync.dma_start(out=idt[:], in_=idm[:])

            def vtile(name):
                return prep.tile([NP128, NF], F32, tag=name, name=name)

            # Rotated coordinates (R = Rx(inc) @ Rz(rot)); rx/ry legs on
            # VectorE (critical path), rz/intensity leg on GpSimd.
            rx, ry, rz = vtile("rx"), vtile("ry"), vtile("rz")
            if PREP_FUSE_STT:
                ya, yb, yc = vtile("ya"), vtile("yb"), vtile("yc")
                t3 = vtile("t3")
                nc.vector.tensor_scalar_mul(ya[:], yt[:], -sr)
                nc.vector.scalar_tensor_tensor(rx[:], xt[:], cr, ya[:],
                                               OP.mult, OP.add)
                nc.vector.tensor_scalar_mul(yb[:], yt[:], ci * cr)
                nc.vector.scalar_tensor_tensor(t3[:], xt[:], ci * sr, yb[:],
                                               OP.mult, OP.add)
                nc.vector.scalar_tensor_tensor(ry[:], zt[:], -si, t3[:],
                                               OP.mult, OP.add)
                nc.gpsimd.tensor_scalar_mul(yc[:], yt[:], si * cr)
                t5 = vtile("t5")
                nc.gpsimd.scalar_tensor_tensor(t5[:], xt[:], si * sr, yc[:],
                                               OP.mult, OP.add)
                nc.gpsimd.scalar_tensor_tensor(rz[:], zt[:], ci, t5[:],
                                               OP.mult, OP.add)
            else:
                xa, ya = vtile("xa"), vtile("ya")
                xb, yb, t3 = vtile("xb"), vtile("yb"), vtile("t3")
                za = vtile("za")
                nc.vector.tensor_scalar_mul(xa[:], xt[:], cr)
                nc.vector.tensor_scalar_mul(ya[:], yt[:], -sr)
                nc.vector.tensor_add(rx[:], xa[:], ya[:])
                nc.vector.tensor_scalar_mul(xb[:], xt[:], ci * sr)
                nc.vector.tensor_scalar_mul(yb[:], yt[:], ci * cr)
                nc.vector.tensor_add(t3[:], xb[:], yb[:])
                nc.vector.tensor_scalar_mul(za[:], zt[:], -si)
                nc.vector.tensor_add(ry[:], t3[:], za[:])
                xc, yc, t5 = vtile("xc"), vtile("yc"), vtile("t5")
                zb = vtile("zb")
                nc.gpsimd.tensor_scalar_mul(xc[:], xt[:], si * sr)
                nc.gpsimd.tensor_scalar_mul(yc[:], yt[:], si * cr)
                nc.gpsimd.tensor_add(t5[:], xc[:], yc[:])
                nc.gpsimd.tensor_scalar_mul(zb[:], zt[:], ci)
                nc.gpsimd.tensor_add(rz[:], t5[:], zb[:])

            # r via exp(0.5*ln(q)) (keeps every activation in the
            # natural_log_exp_and_others table; Sqrt would force a swap).
            sqx, sqy, q, qs = vtile("sqx"), vtile("sqy"), vtile("q"), vtile("qs")
            lnq, r = vtile("lnq"), vtile("r")
            nc.vector.tensor_mul(sqx[:], rx[:], rx[:])
            nc.vector.tensor_mul(sqy[:], ry[:], ry[:])
            nc.vector.tensor_add(q[:], sqy[:], sqx[:])
            nc.vector.tensor_scalar_max(qs[:], q[:], 1e-35)
            nc.scalar.activation(lnq[:], qs[:], AF.Ln)
            nc.scalar.activation(r[:], lnq[:], AF.Exp, scale=0.5)

            # u0 = rx*tanh(r/2)/r = rx*(e^r-1) / (r*(e^r+1))
            er = vtile("er")
            den, rec, t1, u0 = (vtile("den"), vtile("rec"), vtile("t1"),
                                vtile("u0"))
            nc.scalar.activation(er[:], r[:], AF.Exp)
            if PREP_FUSE_STT:
                nc.vector.scalar_tensor_tensor(den[:], er[:], 1.0, r[:],
                                               OP.add, OP.mult)
                nc.vector.reciprocal(rec[:], den[:])
                nc.gpsimd.scalar_tensor_tensor(t1[:], er[:], -1.0, rx[:],
                                               OP.add, OP.mult)
                nc.vector.tensor_mul(u0[:], t1[:], rec[:])
            else:
                ed, num = vtile("ed"), vtile("num")
                nc.vector.tensor_scalar_add(ed[:], er[:], 1.0)
                nc.vector.tensor_mul(den[:], ed[:], r[:])
                nc.vector.reciprocal(rec[:], den[:])
                nc.gpsimd.tensor_scalar_add(num[:], er[:], -1.0)
                nc.gpsimd.tensor_mul(t1[:], rx[:], num[:])
                nc.vector.tensor_mul(u0[:], t1[:], rec[:])

            # A = L + lnnorm - (u/sig)^2 ; L = -r/3 - 2|rz| ; B = 2u/sig^2
            az, rterm, Lt = vtile("az"), vtile("rterm"), vtile("Lt")
            s1, ssq, At, Bt = (vtile("s1"), vtile("ssq"), vtile("At"),
                               vtile("Bt"))
            nc.scalar.activation(az[:], rz[:], AF.Abs)
            nc.gpsimd.tensor_scalar(rterm[:], r[:], -1.0 / 3.0, lnnorm,
                                    OP.mult, OP.add)
            if PREP_FUSE_STT:
                nc.gpsimd.scalar_tensor_tensor(Lt[:], az[:], -2.0, rterm[:],
                                               OP.mult, OP.add)
            else:
                azs = vtile("azs")
                nc.gpsimd.tensor_scalar_mul(azs[:], az[:], -2.0)
                nc.gpsimd.tensor_add(Lt[:], azs[:], rterm[:])
            nc.vector.tensor_scalar_mul(s1[:], u0[:], usc / math.sqrt(sig2))
            nc.vector.tensor_mul(ssq[:], s1[:], s1[:])
            nc.vector.tensor_sub(At[:], Lt[:], ssq[:])
            nc.vector.tensor_scalar_mul(Bt[:], u0[:], usc * 2.0 / sig2)

            if not safe_affine:
                P0t = vtile("P0t")
                nc.scalar.activation(P0t[:], At[:], AF.Exp)
                P0b = prep.tile([NP128, NF], BF16, tag="P0b", name="P0b")
                nc.vector.tensor_copy(P0b[:], P0t[:])

            # KDE over NC coarse bins; Op[t, 4b+m] accumulates the z-sums
            # (pixel p=4t+m of bin b) via three selector matmuls per bin
            # (Ew stationary, tiny selector moving: PSUM base partition 0).
            Op = psum.tile([NT, 4 * NC], F32)
            for b in range(NC):
                vv = float(vc[b])
                if not safe_affine:
                    e1 = kde.tile([NP128, NT, 3], BF16, tag="e1", bufs=4)
                    nc.scalar.activation(e1[:], Bt[:], AF.Exp, scale=vv,
                                         bias=bct[:, b:b + 1])
                    Ew = kde.tile([NP128, NT, 3], BF16, tag="Ew", bufs=4)
                    eng = nc.gpsimd if b % (NC // max(KDE_POOL_MULTS, 1)) == 0 \
                        and KDE_POOL_MULTS > 0 else nc.vector
                    eng.tensor_mul(Ew[:], e1[:], P0b[:])
                else:
                    # overflow-safe path: arg = (B*vv + c_v) + A <= lnnorm
                    cvb = float(-vc[b] * vc[b] / sig2)
                    tmp = kde.tile([NP128, NF], F32, tag="tmpa", bufs=3)
                    nc.vector.tensor_scalar(tmp[:], Bt[:], vv, cvb,
                                            OP.mult, OP.add)
                    arg = kde.tile([NP128, NF], F32, tag="arga", bufs=3)
                    nc.vector.tensor_add(arg[:], tmp[:], At[:])
                    Ew = kde.tile([NP128, NT, 3], BF16, tag="Ew", bufs=4)
                    nc.scalar.activation(Ew[:], arg[:], AF.Exp)
                for c in range(3):
                    nc.tensor.matmul(Op[:, 4 * b:4 * b + 4],
                                     Ew[:, :, c],
                                     sct[:, 4 * c:4 * c + 4],
                                     start=(c == 0), stop=(c == 2))

            # rearrange Op[t, (b,m)] -> Os[(b,m), t] via a PE transpose
            Ops = io.tile([NT, 4 * NC], BF16, tag="Ops")
            nc.vector.tensor_copy(Ops[:], Op[:])
            Tp = psum.tile([4 * NC, NT], BF16)
            nc.tensor.transpose(Tp[:], Ops[:], idt[:])
            # cube2[px=4t+m, v] = sum_b Os[(b,m), t] * R[b, v]
            Os = io.tile([NP128, NT], BF16, tag="Os")
            nc.vector.tensor_copy(Os[0:4 * NC, :], Tp[:])
            if 4 * NC < NP128:
                nc.vector.memset(Os[4 * NC:NP128, :], 0.0)
            out1 = psum.tile([NT, 4 * NV], F32)
            nc.tensor.matmul(out1[:], Os[:], wrt[:], start=True, stop=True)
            Os1 = io.tile([NT, 4 * NV], BF16, tag="Os1")
            nc.vector.tensor_copy(Os1[:], out1[:])

            # j-downsample: outf[jj, (i,v)] = sum_j wj[j,jj] cube2[96i+j, v]
            outf = psum.tile([OUT_J, ROWS_PER_CORE * NV], F32)
            for i in range(ROWS_PER_CORE):
                for m in range(4):
                    nc.tensor.matmul(outf[:, NV * i:NV * (i + 1)],
                                     smt[:, (i * 4 + m) * OUT_J:
                                         (i * 4 + m + 1) * OUT_J],
                                     Os1[:, NV * m:NV * (m + 1)],
                                     start=(m == 0), stop=(m == 3))
            outf_sb = io.tile([OUT_J, ROWS_PER_CORE * NV], F32, tag="outf_sb")
            nc.vector.tensor_copy(outf_sb[:], outf[:])
            nc.sync.dma_start(out=out[:], in_=outf_sb[:])

    return nc


def _recon_matrix(vel, sig2, si):
    """Ridge-regularized reconstruction R[NC, NV]: coarse Gaussian samples
    -> fine samples, fit over all reachable centers u."""
    vel = np.asarray(vel, np.float64).reshape(-1)
    vc = np.linspace(float(vel.min()), float(vel.max()), NC)
    umax = max(200.0 * abs(si), 1e-3)
    uu = np.linspace(-umax * 1.02, umax * 1.02, 4001)
    Ac = np.exp(-((vc[None, :] - uu[:, None]) ** 2) / sig2)
    Af = np.exp(-((vel[None, :] - uu[:, None]) ** 2) / sig2)
    R = np.linalg.solve(Ac.T @ Ac + RIDGE_LAM * np.eye(NC), Ac.T @ Af)
    return R.astype(np.float32)


def kernel(**inputs):
    inc = float(np.asarray(inputs["inclination"]).reshape(-1)[0])
    rot = float(np.asarray(inputs["sky_rot"]).reshape(-1)[0])
    lb = float(np.asarray(inputs["line_broadening"]).reshape(-1)[0])
    vel = np.asarray(inputs["velocity_grid"], np.float32).reshape(-1)
    X = np.asarray(inputs["Xgrid"], np.float32)
    Y = np.asarray(inputs["Ygrid"], np.float32)
    Z = np.asarray(inputs["Zgrid"], np.float32)

    ci, si = math.cos(inc), math.sin(inc)
    cr, sr = math.cos(rot), math.sin(rot)
    sig2 = float(np.float32(lb) * np.float32(lb))
    if not (sig2 > 0.0) or not math.isfinite(sig2):
        sig2 = 1e-30  # degenerate sigma: reference output is ~0/NaN anyway
    lnnorm = float(-0.5 * math.log(2.0 * math.pi * sig2))

    nc = _build_program(ci, si, cr, sr, sig2, lnnorm, vel)
    nc.finalize()

    vc = np.linspace(float(vel.min()), float(vel.max()), NC)
    bcv = np.ascontiguousarray(
        np.tile((-(vc.astype(np.float64) ** 2) / sig2).astype(np.float32),
                (NP128, 1)))

    # selector stationaries S_c
    scv = np.zeros((NP128, 12), np.float32)
    for c in range(3):
        for k in range(NP128):
            m = (128 * c + k) // 96
            if 0 <= m < 4 and 96 * m <= 128 * c + k < 96 * (m + 1):
                scv[k, 4 * c + m] = 1.0

    # reconstruction moving matrix W[(b,m), (m',v)] = delta R[b, v]
    R = _recon_matrix(vel, sig2, si)
    wrv = np.zeros((NP128, 4 * NV), np.float32)
    for b in range(NC):
        for m in range(4):
            wrv[4 * b + m, NV * m:NV * (m + 1)] = R[b]

    # j-downsample stencil and its zero-padded stationaries
    wj = np.zeros((G, OUT_J), np.float32)
    for m in range(OUT_J // 2):
        wj[3 * m, 2 * m] = 0.75
        wj[3 * m + 1, 2 * m] = 0.25
        wj[3 * m + 1, 2 * m + 1] = 0.25
        wj[3 * m + 2, 2 * m + 1] = 0.75
    smv = np.zeros((NT, 16 * OUT_J), np.float32)
    for i in range(4):
        for m in range(4):
            col = (i * 4 + m) * OUT_J
            for s in range(24):
                smv[24 * i + s, col:col + OUT_J] = wj[4 * s + m]

    as_bf16 = (lambda a: np.ascontiguousarray(a.astype(_BF16))) if _BF16 \
        else (lambda a: np.ascontiguousarray(a))

    in_maps = []
    for c in range(N_CORES):
        rows = [3 * k + 1 for k in range(ROWS_PER_CORE * c,
                                         ROWS_PER_CORE * (c + 1))]
        def shard(a):
            s = a[rows]                      # (4, 96, 96) = (i, j, z)
            flat = s.reshape(-1)             # flat = px*96 + z
            t = flat.reshape(NF, NP128).T    # [partition, free]
            return np.ascontiguousarray(t)
        in_maps.append({"xs": shard(X), "ys": shard(Y), "zs": shard(Z),
                        "bc": bcv, "sc": as_bf16(scv), "wr": as_bf16(wrv),
                        "sm": as_bf16(smv), "idm": as_bf16(np.eye(NT, dtype=np.float32))})

    res = run_bass_kernel_spmd(nc, in_maps, core_ids=list(range(N_CORES)))
    global LAST_EXEC_NS
    LAST_EXEC_NS = res.exec_time_ns

    parts = []
    for c in range(N_CORES):
        o = res.results[c]["out"]            # (64, 256) = [jj, i*64+v]
        parts.append(o.reshape(OUT_J, ROWS_PER_CORE, NV).transpose(1, 0, 2))
    return np.concatenate(parts, axis=0).astype(np.float32)  # (32, 64, 64)


# revision 7
# speedup vs baseline: 1.4996x; 1.0666x over previous
"""Trainium2 Bass kernel for the CubeSimulator problem (v2).

Reference: rotate (96,96,96) grids, per-voxel line-of-sight velocity u and
intensity I, Gaussian-KDE cube over 64 velocity bins, then trilinear
downsample (96,96,64) -> (32,64,64).

Exact structure reused from v1 (validated):
 - axis0 downsample (96->32) is a pure selection of rows 3k+1;
 - axis2 downsample (64->64) is the identity;
 - axis1 downsample (96->64) is a 2-tap stencil (0.75/0.25) matmul;
 - exp(L - (v-u)^2/sig^2) = exp(A) * exp(v*B + c_v) with
   A = L + ln(norm) - u^2/sig^2, B = 2u/sig^2, c_v = -v^2/sig^2.

New in v2 (tolerance-aware, rel err ~1.5e-3 vs 2e-2 budget):
 - Coarse-bin KDE: the cube is computed at NC=32 velocity centers and all
   64 reference bins are reconstructed with a ridge-regularized
   least-squares matrix R (a Gaussian with sigma=30 sampled at dv=19 is
   ~3x oversampled; aliasing ~2e-4). Halves the dominant per-bin work.
 - Wrap layout [128, 288]: per-core voxels flat=(px*96+z) are laid out
   partition=flat%128, free=flat//128, using all 128 lanes (elementwise
   engine cost scales with free size only). The z-sum for pixel p covers
   flat [96p, 96p+96), reduced on the (otherwise idle) TensorE with three
   accumulating matmuls per bin whose [128,4] selector stationaries are
   independent of the column triplet (128*3 = 96*4).
 - KDE tiles in bf16: DVE runs 2-byte tensor_tensor at 2x; exp args stay
   fp32 (ACT reads Bt fp32; scale/bias are per-bin immediates/APs).
 - Per-bin path: one ScalarE Exp (scale=vc, bias=c_v AP) + one V/P mult
   by P0 = exp(A). ScalarE is the bottleneck engine; all of prep's
   square/abs/ln/exp stay inside the natural_log_exp_and_others table.

Sharding: 32 needed i-rows split 4-per-core across 8 cores; only the
final (64, 4*64) tile is gathered per core.
"""

import math

import numpy as np

import concourse.bacc as bacc
import concourse.bass as bass
import concourse.mybir as mybir
import concourse.tile as tile
from concourse.bass_utils import run_bass_kernel_spmd

try:
    import ml_dtypes
    _BF16 = np.dtype(ml_dtypes.bfloat16)
except Exception:  # pragma: no cover
    _BF16 = None

G = 96            # up_gal grid size
NV = 64           # reference velocity bins
NC = 32           # coarse KDE bins (reconstructed to NV by matmul)
N_CORES = 8
OUT_I = 32        # selected i rows (axis-0 downsample = row selection)
ROWS_PER_CORE = OUT_I // N_CORES   # 4
PX = ROWS_PER_CORE * G             # 384 pixels per core
NZ = G                             # z depth
NFLAT = PX * NZ                    # 36864 voxels per core
NP128 = 128
NF = NFLAT // NP128                # 288 free columns
NT = NF // 3                       # 96 column triplets (4 pixels each)
OUT_J = 64
RIDGE_LAM = 1e-4

F32 = mybir.dt.float32
BF16 = mybir.dt.bfloat16
AF = mybir.ActivationFunctionType
OP = mybir.AluOpType

LAST_EXEC_NS = None

# tuning knobs
KDE_POOL_MULTS = 0   # of the NC bf16 P0-mults, how many go to GpSimd
PREP_FUSE_STT = False # use scalar_tensor_tensor fusion in prep where legal


def _build_program(ci, si, cr, sr, sig2, lnnorm, vel, safe_affine=None):
    vel = np.asarray(vel, np.float64).reshape(-1)
    vc = np.linspace(float(vel.min()), float(vel.max()), NC)
    usc = -200.0 * si
    if safe_affine is None:
        umax2 = (200.0 * si) ** 2
        safe_affine = not (umax2 / sig2 <= 80.0)

    nc = bacc.Bacc("TRN2")

    xs = nc.dram_tensor("xs", [NP128, NF], F32, kind="ExternalInput")
    ys = nc.dram_tensor("ys", [NP128, NF], F32, kind="ExternalInput")
    zs = nc.dram_tensor("zs", [NP128, NF], F32, kind="ExternalInput")
    # per-coarse-bin exp biases c_v = -vc^2/sig^2 (replicated on partitions)
    bc = nc.dram_tensor("bc", [NP128, NC], F32, kind="ExternalInput")
    # z-reduce selector stationaries S_c[k, m] = 1{96m <= 128c+k < 96m+96}
    sc = nc.dram_tensor("sc", [NP128, 12], BF16, kind="ExternalInput")
    # reconstruction moving matrix W[(b,m), (m',v)] = delta_{m,m'} R[b, v]
    wr = nc.dram_tensor("wr", [NP128, 4 * NV], BF16, kind="ExternalInput")
    # j-downsample stationaries, zero-padded to 96 partitions:
    # sm[:, (i*4+m)*64 + jj] = wj[4s+m, jj] on rows t=24i+s, else 0
    sm = nc.dram_tensor("sm", [NT, 16 * OUT_J], BF16, kind="ExternalInput")
    # identity for the PE transpose
    idm = nc.dram_tensor("idm", [NT, NT], BF16, kind="ExternalInput")
    out = nc.dram_tensor("out", [OUT_J, ROWS_PER_CORE * NV], F32,
                         kind="ExternalOutput")

    with tile.TileContext(nc) as tc:
        with (
            tc.tile_pool(name="io", bufs=1) as io,
            tc.tile_pool(name="prep", bufs=1) as prep,
            tc.tile_pool(name="kde", bufs=2) as kde,
            tc.tile_pool(name="psum", bufs=1, space="PSUM") as psum,
        ):
            # Preload the one activation table that covers every func
            # used (ln/exp/abs): avoids two mid-kernel table swaps (~1.3us
            # each) that the auto-inserter's minimal-set choice would cause.
            from concourse.hw_specs import get_activation_tables
            tabs = get_activation_tables(nc.m.arch)
            want = {AF.Ln, AF.Exp, AF.Abs}
            set_id = None
            for idx, (tname, funcs) in enumerate(tabs.items()):
                if want.issubset(funcs):
                    set_id = idx
                    break
            if set_id is not None:
                ld = mybir.InstLoadActFuncSet(
                    name=nc.scalar.bass.get_next_instruction_name(),
                    act_func_set_id=set_id, ins=[], outs=[])
                nc.scalar.add_instruction(ld)

            xt = io.tile([NP128, NF], F32, tag="xt")
            yt = io.tile([NP128, NF], F32, tag="yt")
            zt = io.tile([NP128, NF], F32, tag="zt")
            nc.sync.dma_start(out=xt[:], in_=xs[:])
            nc.sync.dma_start(out=yt[:], in_=ys[:])
            nc.sync.dma_start(out=zt[:], in_=zs[:])
            bct = io.tile([NP128, NC], F32, tag="bct")
            nc.sync.dma_start(out=bct[:], in_=bc[:])
            sct = io.tile([NP128, 12], BF16, tag="sct")
            nc.sync.dma_start(out=sct[:], in_=sc[:])
            wrt = io.tile([NP128, 4 * NV], BF16, tag="wrt")
            nc.sync.dma_start(out=wrt[:], in_=wr[:])
            smt = io.tile([NT, 16 * OUT_J], BF16, tag="smt")
            nc.sync.dma_start(out=smt[:], in_=sm[:])
            idt = io.tile([NT, NT], BF16, tag="idt")
            nc.sync.dma_start(out=idt[:], in_=idm[:])

            def vtile(name):
                return prep.tile([NP128, NF], F32, tag=name, name=name)

            # Rotated coordinates (R = Rx(inc) @ Rz(rot)); rx/ry legs on
            # VectorE (critical path), rz/intensity leg on GpSimd.
            rx, ry, rz = vtile("rx"), vtile("ry"), vtile("rz")
            if PREP_FUSE_STT:
                ya, yb, yc = vtile("ya"), vtile("yb"), vtile("yc")
                t3 = vtile("t3")
                nc.vector.tensor_scalar_mul(ya[:], yt[:], -sr)
                nc.vector.scalar_tensor_tensor(rx[:], xt[:], cr, ya[:],
                                               OP.mult, OP.add)
                nc.vector.tensor_scalar_mul(yb[:], yt[:], ci * cr)
                nc.vector.scalar_tensor_tensor(t3[:], xt[:], ci * sr, yb[:],
                                               OP.mult, OP.add)
                nc.vector.scalar_tensor_tensor(ry[:], zt[:], -si, t3[:],
                                               OP.mult, OP.add)
                nc.gpsimd.tensor_scalar_mul(yc[:], yt[:], si * cr)
                t5 = vtile("t5")
                nc.gpsimd.scalar_tensor_tensor(t5[:], xt[:], si * sr, yc[:],
                                               OP.mult, OP.add)
                nc.gpsimd.scalar_tensor_tensor(rz[:], zt[:], ci, t5[:],
                                               OP.mult, OP.add)
            else:
                xa, ya = vtile("xa"), vtile("ya")
                xb, yb, t3 = vtile("xb"), vtile("yb"), vtile("t3")
                za = vtile("za")
                nc.vector.tensor_scalar_mul(xa[:], xt[:], cr)
                nc.vector.tensor_scalar_mul(ya[:], yt[:], -sr)
                nc.vector.tensor_add(rx[:], xa[:], ya[:])
                nc.vector.tensor_scalar_mul(xb[:], xt[:], ci * sr)
                nc.vector.tensor_scalar_mul(yb[:], yt[:], ci * cr)
                nc.vector.tensor_add(t3[:], xb[:], yb[:])
                nc.vector.tensor_scalar_mul(za[:], zt[:], -si)
                nc.vector.tensor_add(ry[:], t3[:], za[:])
                xc, yc, t5 = vtile("xc"), vtile("yc"), vtile("t5")
                zb = vtile("zb")
                nc.gpsimd.tensor_scalar_mul(xc[:], xt[:], si * sr)
                nc.gpsimd.tensor_scalar_mul(yc[:], yt[:], si * cr)
                nc.gpsimd.tensor_add(t5[:], xc[:], yc[:])
                nc.gpsimd.tensor_scalar_mul(zb[:], zt[:], ci)
                nc.gpsimd.tensor_add(rz[:], t5[:], zb[:])

            # r via exp(0.5*ln(q)) (keeps every activation in the
            # natural_log_exp_and_others table; Sqrt would force a swap).
            sqx, sqy, q, qs = vtile("sqx"), vtile("sqy"), vtile("q"), vtile("qs")
            lnq, r = vtile("lnq"), vtile("r")
            nc.vector.tensor_mul(sqx[:], rx[:], rx[:])
            nc.vector.tensor_mul(sqy[:], ry[:], ry[:])
            nc.vector.tensor_add(q[:], sqy[:], sqx[:])
            nc.vector.tensor_scalar_max(qs[:], q[:], 1e-35)
            nc.scalar.activation(lnq[:], qs[:], AF.Ln)
            nc.scalar.activation(r[:], lnq[:], AF.Exp, scale=0.5)

            # u0 = rx*tanh(r/2)/r = rx*(e^r-1) / (r*(e^r+1))
            er = vtile("er")
            den, rec, t1, u0 = (vtile("den"), vtile("rec"), vtile("t1"),
                                vtile("u0"))
            nc.scalar.activation(er[:], r[:], AF.Exp)
            if PREP_FUSE_STT:
                nc.vector.scalar_tensor_tensor(den[:], er[:], 1.0, r[:],
                                               OP.add, OP.mult)
                nc.vector.reciprocal(rec[:], den[:])
                nc.gpsimd.scalar_tensor_tensor(t1[:], er[:], -1.0, rx[:],
                                               OP.add, OP.mult)
                nc.vector.tensor_mul(u0[:], t1[:], rec[:])
            else:
                ed, num = vtile("ed"), vtile("num")
                nc.vector.tensor_scalar_add(ed[:], er[:], 1.0)
                nc.vector.tensor_mul(den[:], ed[:], r[:])
                nc.vector.reciprocal(rec[:], den[:])
                nc.gpsimd.tensor_scalar_add(num[:], er[:], -1.0)
                nc.vector.tensor_mul(t1[:], rx[:], num[:])
                nc.vector.tensor_mul(u0[:], t1[:], rec[:])

            # A = L + lnnorm - (u/sig)^2 ; L = -r/3 - 2|rz| ; B = 2u/sig^2
            az, rterm, Lt = vtile("az"), vtile("rterm"), vtile("Lt")
            s1, ssq, At, Bt = (vtile("s1"), vtile("ssq"), vtile("At"),
                               vtile("Bt"))
            nc.scalar.activation(az[:], rz[:], AF.Abs)
            nc.gpsimd.tensor_scalar(rterm[:], r[:], -1.0 / 3.0, lnnorm,
                                    OP.mult, OP.add)
            if PREP_FUSE_STT:
                nc.gpsimd.scalar_tensor_tensor(Lt[:], az[:], -2.0, rterm[:],
                                               OP.mult, OP.add)
            else:
                azs = vtile("azs")
                nc.gpsimd.tensor_scalar_mul(azs[:], az[:], -2.0)
                nc.gpsimd.tensor_add(Lt[:], azs[:], rterm[:])
            nc.vector.tensor_scalar_mul(Bt[:], u0[:], usc * 2.0 / sig2)
            nc.vector.tensor_scalar_mul(s1[:], u0[:], usc / math.sqrt(sig2))
            nc.vector.tensor_mul(ssq[:], s1[:], s1[:])
            nc.vector.tensor_sub(At[:], Lt[:], ssq[:])

            if not safe_affine:
                P0t = vtile("P0t")
                nc.scalar.activation(P0t[:], At[:], AF.Exp)
                P0b = prep.tile([NP128, NF], BF16, tag="P0b", name="P0b")
                nc.vector.tensor_copy(P0b[:], P0t[:])

            # KDE over NC coarse bins; Op[t, 4b+m] accumulates the z-sums
            # (pixel p=4t+m of bin b) via three selector matmuls per bin
            # (Ew stationary, tiny selector moving: PSUM base partition 0).
            Op = psum.tile([NT, 4 * NC], F32)
            for b in range(NC):
                vv = float(vc[b])
                if not safe_affine:
                    e1 = kde.tile([NP128, NT, 3], BF16, tag="e1", bufs=6)
                    nc.scalar.activation(e1[:], Bt[:], AF.Exp, scale=vv,
                                         bias=bct[:, b:b + 1])
                    Ew = kde.tile([NP128, NT, 3], BF16, tag="Ew", bufs=6)
                    eng = nc.gpsimd if b % (NC // max(KDE_POOL_MULTS, 1)) == 0 \
                        and KDE_POOL_MULTS > 0 else nc.vector
                    eng.tensor_mul(Ew[:], e1[:], P0b[:])
                else:
                    # overflow-safe path: arg = (B*vv + c_v) + A <= lnnorm
                    cvb = float(-vc[b] * vc[b] / sig2)
                    tmp = kde.tile([NP128, NF], F32, tag="tmpa", bufs=3)
                    nc.vector.tensor_scalar(tmp[:], Bt[:], vv, cvb,
                                            OP.mult, OP.add)
                    arg = kde.tile([NP128, NF], F32, tag="arga", bufs=3)
                    nc.vector.tensor_add(arg[:], tmp[:], At[:])
                    Ew = kde.tile([NP128, NT, 3], BF16, tag="Ew", bufs=6)
                    nc.scalar.activation(Ew[:], arg[:], AF.Exp)
                for c in range(3):
                    nc.tensor.matmul(Op[:, 4 * b:4 * b + 4],
                                     Ew[:, :, c],
                                     sct[:, 4 * c:4 * c + 4],
                                     start=(c == 0), stop=(c == 2))

            # rearrange Op[t, (b,m)] -> Os[(b,m), t] via a PE transpose
            Ops = io.tile([NT, 4 * NC], BF16, tag="Ops")
            nc.vector.tensor_copy(Ops[:], Op[:])
            Tp = psum.tile([4 * NC, NT], BF16)
            nc.tensor.transpose(Tp[:], Ops[:], idt[:])
            # cube2[px=4t+m, v] = sum_b Os[(b,m), t] * R[b, v]
            Os = io.tile([NP128, NT], BF16, tag="Os")
            nc.vector.tensor_copy(Os[0:4 * NC, :], Tp[:])
            if 4 * NC < NP128:
                nc.vector.memset(Os[4 * NC:NP128, :], 0.0)
            out1 = psum.tile([NT, 4 * NV], F32)
            nc.tensor.matmul(out1[:], Os[:], wrt[:], start=True, stop=True)
            Os1 = io.tile([NT, 4 * NV], BF16, tag="Os1")
            nc.vector.tensor_copy(Os1[:], out1[:])

            # j-downsample: outf[jj, (i,v)] = sum_j wj[j,jj] cube2[96i+j, v]
            outf = psum.tile([OUT_J, ROWS_PER_CORE * NV], F32)
            for i in range(ROWS_PER_CORE):
                for m in range(4):
                    nc.tensor.matmul(outf[:, NV * i:NV * (i + 1)],
                                     smt[:, (i * 4 + m) * OUT_J:
                                         (i * 4 + m + 1) * OUT_J],
                                     Os1[:, NV * m:NV * (m + 1)],
                                     start=(m == 0), stop=(m == 3))
            outf_sb = io.tile([OUT_J, ROWS_PER_CORE * NV], F32, tag="outf_sb")
            nc.vector.tensor_copy(outf_sb[:], outf[:])
            nc.sync.dma_start(out=out[:], in_=outf_sb[:])

    return nc


def _recon_matrix(vel, sig2, si):
    """Ridge-regularized reconstruction R[NC, NV]: coarse Gaussian samples
    -> fine samples, fit over all reachable centers u."""
    vel = np.asarray(vel, np.float64).reshape(-1)
    vc = np.linspace(float(vel.min()), float(vel.max()), NC)
    umax = max(200.0 * abs(si), 1e-3)
    uu = np.linspace(-umax * 1.02, umax * 1.02, 4001)
    Ac = np.exp(-((vc[None, :] - uu[:, None]) ** 2) / sig2)
    Af = np.exp(-((vel[None, :] - uu[:, None]) ** 2) / sig2)
    R = np.linalg.solve(Ac.T @ Ac + RIDGE_LAM * np.eye(NC), Ac.T @ Af)
    return R.astype(np.float32)


def kernel(**inputs):
    inc = float(np.asarray(inputs["inclination"]).reshape(-1)[0])
    rot = float(np.asarray(inputs["sky_rot"]).reshape(-1)[0])
    lb = float(np.asarray(inputs["line_broadening"]).reshape(-1)[0])
    vel = np.asarray(inputs["velocity_grid"], np.float32).reshape(-1)
    X = np.asarray(inputs["Xgrid"], np.float32)
    Y = np.asarray(inputs["Ygrid"], np.float32)
    Z = np.asarray(inputs["Zgrid"], np.float32)

    ci, si = math.cos(inc), math.sin(inc)
    cr, sr = math.cos(rot), math.sin(rot)
    sig2 = float(np.float32(lb) * np.float32(lb))
    if not (sig2 > 0.0) or not math.isfinite(sig2):
        sig2 = 1e-30  # degenerate sigma: reference output is ~0/NaN anyway
    lnnorm = float(-0.5 * math.log(2.0 * math.pi * sig2))

    nc = _build_program(ci, si, cr, sr, sig2, lnnorm, vel)
    nc.finalize()

    vc = np.linspace(float(vel.min()), float(vel.max()), NC)
    bcv = np.ascontiguousarray(
        np.tile((-(vc.astype(np.float64) ** 2) / sig2).astype(np.float32),
                (NP128, 1)))

    # selector stationaries S_c
    scv = np.zeros((NP128, 12), np.float32)
    for c in range(3):
        for k in range(NP128):
            m = (128 * c + k) // 96
            if 0 <= m < 4 and 96 * m <= 128 * c + k < 96 * (m + 1):
                scv[k, 4 * c + m] = 1.0

    # reconstruction moving matrix W[(b,m), (m',v)] = delta R[b, v]
    R = _recon_matrix(vel, sig2, si)
    wrv = np.zeros((NP128, 4 * NV), np.float32)
    for b in range(NC):
        for m in range(4):
            wrv[4 * b + m, NV * m:NV * (m + 1)] = R[b]

    # j-downsample stencil and its zero-padded stationaries
    wj = np.zeros((G, OUT_J), np.float32)
    for m in range(OUT_J // 2):
        wj[3 * m, 2 * m] = 0.75
        wj[3 * m + 1, 2 * m] = 0.25
        wj[3 * m + 1, 2 * m + 1] = 0.25
        wj[3 * m + 2, 2 * m + 1] = 0.75
    smv = np.zeros((NT, 16 * OUT_J), np.float32)
    for i in range(4):
        for m in range(4):
            col = (i * 4 + m) * OUT_J
            for s in range(24):
                smv[24 * i + s, col:col + OUT_J] = wj[4 * s + m]

    as_bf16 = (lambda a: np.ascontiguousarray(a.astype(_BF16))) if _BF16 \
        else (lambda a: np.ascontiguousarray(a))

    in_maps = []
    for c in range(N_CORES):
        rows = [3 * k + 1 for k in range(ROWS_PER_CORE * c,
                                         ROWS_PER_CORE * (c + 1))]
        def shard(a):
            s = a[rows]                      # (4, 96, 96) = (i, j, z)
            flat = s.reshape(-1)             # flat = px*96 + z
            t = flat.reshape(NF, NP128).T    # [partition, free]
            return np.ascontiguousarray(t)
        in_maps.append({"xs": shard(X), "ys": shard(Y), "zs": shard(Z),
                        "bc": bcv, "sc": as_bf16(scv), "wr": as_bf16(wrv),
                        "sm": as_bf16(smv), "idm": as_bf16(np.eye(NT, dtype=np.float32))})

    res = run_bass_kernel_spmd(nc, in_maps, core_ids=list(range(N_CORES)))
    global LAST_EXEC_NS
    LAST_EXEC_NS = res.exec_time_ns

    parts = []
    for c in range(N_CORES):
        o = res.results[c]["out"]            # (64, 256) = [jj, i*64+v]
        parts.append(o.reshape(OUT_J, ROWS_PER_CORE, NV).transpose(1, 0, 2))
    return np.concatenate(parts, axis=0).astype(np.float32)  # (32, 64, 64)
